# revision 1
# baseline (speedup 1.0000x reference)
"""Pointer-generator copy layer on 8 Trainium2 NeuronCores.

Math per row r=(b,t):
  p      = sigmoid(x_r @ w_gen + b_gen)
  logits = x_r @ W_vocab + b_vocab                  # V=32000
  E      = exp(logits);  S = sum(E)                 # softmax denom (no max-sub:
                                                    #  logits ~ N(0,1), safe)
  cd     = (1-p) * softmax(attn_r)                  # L=512
  corr   = cd @ onehot(enc_b)                       # scatter-add as matmul
  out    = log(E * (p/S) + corr)

Default sharding ("vocab"): tensor-parallel over the vocab dim. Each core owns
a 4000-wide W_vocab shard and all 800 rows; the softmax denominator S is
AllReduce-summed across the 8 cores (tiny [800] payload). The scatter-add is a
matmul against a one-hot built ON DEVICE (iota + is_equal against the values
of the <=128 in-shard encoder indices per batch, host-selected; out-of-shard
indices match nothing, which is the per-shard masking). The copy weights are
pre-compressed to those 128 positions with a tiny selection matmul, so the
scatter matmul contracts over K=128 instead of K=512. All PE matmuls run as float32r (full-rate, fp32-accurate): mixing
bf16 and fp32r matmuls on the PE mis-executes on HW (double-accumulation)
while CoreSim is clean, so every PE operand chain is kept float32r, which
also requires memset-free producers for those tiles.

Fallback ("batch"): data-parallel over batch, no collectives, host-built
bf16 one-hot; ~2x slower (full W_vocab DMA per core) but fully independent.
"""

import numpy as np
import ml_dtypes
from contextlib import ExitStack

import concourse.bass as bass
import concourse.mybir as mybir
import concourse.tile as tile
from concourse.bass_utils import run_bass_kernel_spmd

B, T, H, V, L = 8, 100, 1024, 32000, 512
NCORES = 8
NC = 500               # free-dim chunk width (one PSUM bank of f32)
NCHUNKS = V // NC      # 64
KH = H // 128          # 8
KL = L // 128          # 4
P = 128

F32 = mybir.dt.float32
F32R = mybir.dt.float32r
BF16 = mybir.dt.bfloat16
AF = mybir.ActivationFunctionType
ALU = mybir.AluOpType

_cache = {}


def _legalize_waits(nc, limit=1):
    """This walrus build accepts at most one sync-wait per instruction; the
    TileContext exit drain can carry several. Split extras onto NoOps."""
    for bb in nc.main_func.blocks:
        new_insts = []
        for ins in bb.instructions:
            si = ins.sync_info
            if si is not None and si.on_wait and len(si.on_wait) > limit:
                waits = list(si.on_wait)
                extra, keep = waits[:-limit], waits[-limit:]
                for k, w in enumerate(extra):
                    new_insts.append(
                        mybir.InstNoOp(
                            name=f"{ins.name}-ws{k}",
                            sync_info=mybir.SyncInfo(on_wait=[w], on_update=[]),
                            bass_nofuse=True,
                            engine=ins.engine,
                        )
                    )
                ins.sync_info = mybir.SyncInfo(
                    on_wait=keep, on_update=list(si.on_update)
                )
            new_insts.append(ins)
        bb.instructions[:] = new_insts
    return nc


def _build_batch_sharded(has_bias: bool):
    nc = bass.Bass()
    xT = nc.dram_tensor("xT", [H, T], F32, kind="ExternalInput")
    w = nc.dram_tensor("w", [H, V], F32, kind="ExternalInput")
    attn = nc.dram_tensor("attn", [T, L], F32, kind="ExternalInput")
    ohT = nc.dram_tensor("ohT", [L, V], BF16, kind="ExternalInput")
    wg = nc.dram_tensor("wg", [H, 2], F32, kind="ExternalInput")
    bg = nc.dram_tensor("bg", [P, 1], F32, kind="ExternalInput")
    bv = nc.dram_tensor("bv", [P, V], F32, kind="ExternalInput") if has_bias else None
    out = nc.dram_tensor("out", [T, V], F32, kind="ExternalOutput")

    with ExitStack() as ctx:
        tc = ctx.enter_context(tile.TileContext(nc))
        const = ctx.enter_context(tc.tile_pool(name="const", bufs=1))
        wpool = ctx.enter_context(tc.tile_pool(name="wp", bufs=2))
        ohpool = ctx.enter_context(tc.tile_pool(name="ohp", bufs=3))
        epool = ctx.enter_context(tc.tile_pool(name="ep", bufs=1))
        psl = ctx.enter_context(tc.tile_pool(name="psl", bufs=4, space="PSUM"))
        psc = ctx.enter_context(tc.tile_pool(name="psc", bufs=3, space="PSUM"))
        pss = ctx.enter_context(tc.tile_pool(name="pss", bufs=1, space="PSUM"))
        stg = ctx.enter_context(tc.tile_pool(name="stg", bufs=3))
        stg2 = ctx.enter_context(tc.tile_pool(name="stg2", bufs=3))
        small = ctx.enter_context(tc.tile_pool(name="small", bufs=1))
        bvpool = (
            ctx.enter_context(tc.tile_pool(name="bvp", bufs=2)) if has_bias else None
        )

        # ---- constants / small inputs ----
        xk = const.tile([P, KH, T], F32R)
        nc.sync.dma_start(
            xk[:], xT[:, :].rearrange("(k p) t -> p k t", p=P).bitcast(F32R)
        )
        wgt = const.tile([P, KH, 2], F32R)
        nc.sync.dma_start(
            wgt[:], wg[:, :].rearrange("(k p) o -> p k o", p=P).bitcast(F32R)
        )
        bgt = const.tile([P, 1], F32)
        nc.sync.dma_start(bgt[:], bg[:, :])
        attn_t = const.tile([P, L], F32)
        nc.vector.memset(attn_t[:], 0.0)
        nc.sync.dma_start(attn_t[:T, :], attn[:, :])
        ident = const.tile([P, P], BF16)
        from concourse.masks import make_identity

        make_identity(nc, ident[:])

        # ---- p_gen = sigmoid(x @ w_gen + b_gen) ----
        ps_p = pss.tile([P, 2], F32)
        for k in range(KH):
            nc.tensor.matmul(
                ps_p[:T, :2],
                lhsT=xk[:, k, :T],
                rhs=wgt[:, k, :],
                start=(k == 0),
                stop=(k == KH - 1),
            )
        p_sb = small.tile([P, 1], F32)
        nc.scalar.activation(p_sb[:T], ps_p[:T, :1], AF.Sigmoid, bias=bgt[:T])
        q_sb = small.tile([P, 1], F32)  # 1 - p
        nc.scalar.activation(q_sb[:T], p_sb[:T], AF.Identity, bias=1.0, scale=-1.0)

        # ---- copy distribution cd = (1-p) * softmax(attn), bf16, and its T ----
        ea = small.tile([P, L], F32)
        sa = small.tile([P, 1], F32)
        nc.scalar.activation(ea[:T], attn_t[:T], AF.Exp, accum_out=sa[:T])
        ra = small.tile([P, 1], F32)
        nc.vector.reciprocal(ra[:T], sa[:T])
        qr = small.tile([P, 1], F32)  # (1-p)/sum
        nc.vector.tensor_tensor(qr[:T], q_sb[:T], ra[:T], op=ALU.mult)
        cd = small.tile([P, L], BF16)
        nc.vector.memset(cd[:], 0.0)
        nc.vector.tensor_scalar(
            cd[:T], ea[:T], qr[:T, :1], None, op0=ALU.mult
        )
        # transpose cd -> cdT [L, T] via PE (4 blocks of 128)
        cdT = const.tile([P, KL, P], BF16)
        for c in range(KL):
            ps_t = psc.tile([P, P], BF16, tag="c")
            nc.tensor.transpose(ps_t[:], cd[:, bass.ts(c, P)], ident[:])
            nc.vector.tensor_copy(cdT[:, c, :], ps_t[:])

        # ---- phase A: logits chunks -> exp -> E (resident), partial sums ----
        sparts = small.tile([P, NCHUNKS], F32)
        E = epool.tile([P, NCHUNKS, NC], F32)
        for j in range(NCHUNKS):
            wt = wpool.tile([P, KH, NC], F32R)
            nc.sync.dma_start(
                wt[:],
                w[:, bass.ts(j, NC)].rearrange("(k p) n -> p k n", p=P).bitcast(F32R),
            )
            ps_l = psl.tile([P, NC], F32)
            for k in range(KH):
                nc.tensor.matmul(
                    ps_l[:T, :],
                    lhsT=xk[:, k, :T],
                    rhs=wt[:, k, :],
                    start=(k == 0),
                    stop=(k == KH - 1),
                )
            if has_bias:
                bvt = bvpool.tile([P, NC], F32)
                nc.sync.dma_start(bvt[:], bv[:, bass.ts(j, NC)])
                nc.vector.tensor_tensor(ps_l[:T, :], ps_l[:T, :], bvt[:T, :], op=ALU.add)
            nc.scalar.activation(
                E[:T, j, :], ps_l[:T, :], AF.Exp, accum_out=sparts[:T, j : j + 1]
            )

        # ---- softmax scale p/S ----
        S = small.tile([P, 1], F32)
        nc.vector.reduce_sum(S[:T], sparts[:T, :], axis=mybir.AxisListType.X)
        rS = small.tile([P, 1], F32)
        nc.vector.reciprocal(rS[:T], S[:T])
        pscale = small.tile([P, 1], F32)
        nc.vector.tensor_tensor(pscale[:T], p_sb[:T], rS[:T], op=ALU.mult)

        # ---- phase B: corr matmul + combine + log + store ----
        for j in range(NCHUNKS):
            oht = ohpool.tile([P, KL, NC], BF16)
            nc.sync.dma_start(
                oht[:], ohT[:, bass.ts(j, NC)].rearrange("(k p) n -> p k n", p=P)
            )
            ps_c = psc.tile([P, NC], F32, tag="c")
            for c in range(KL):
                nc.tensor.matmul(
                    ps_c[:T, :],
                    lhsT=cdT[:, c, :T],
                    rhs=oht[:, c, :],
                    start=(c == 0),
                    stop=(c == KL - 1),
                )
            comb = stg.tile([P, NC], F32)
            nc.vector.scalar_tensor_tensor(
                comb[:T, :],
                E[:T, j, :],
                pscale[:T, :1],
                ps_c[:T, :],
                op0=ALU.mult,
                op1=ALU.add,
            )
            res = stg2.tile([P, NC], F32)
            nc.scalar.activation(res[:T, :], comb[:T, :], AF.Ln)
            nc.sync.dma_start(out[:, bass.ts(j, NC)], res[:T, :])

    return _legalize_waits(nc)


VS = V // NCORES          # vocab shard width per core (4000)
NCJ = VS // NC            # chunks per shard (8)
RT = B * T                # total rows (800)


def _build_vocab_sharded(has_bias: bool):
    nc = bass.Bass()
    xT = nc.dram_tensor("xT", [H, RT], F32, kind="ExternalInput")
    w = nc.dram_tensor("w", [H, VS], F32, kind="ExternalInput")
    attn = nc.dram_tensor("attn", [RT, L], F32, kind="ExternalInput")
    encsel = nc.dram_tensor("encsel", [P, B], F32, kind="ExternalInput")
    selT = nc.dram_tensor("selT", [B, L, P], F32, kind="ExternalInput")
    wg = nc.dram_tensor("wg", [H, 2], F32, kind="ExternalInput")
    bg = nc.dram_tensor("bg", [P, 1], F32, kind="ExternalInput")
    idn = nc.dram_tensor("idn", [P, P], F32, kind="ExternalInput")
    bv = nc.dram_tensor("bv", [P, VS], F32, kind="ExternalInput") if has_bias else None
    out = nc.dram_tensor("out", [RT, VS], F32, kind="ExternalOutput")

    with ExitStack() as ctx:
        tc = ctx.enter_context(tile.TileContext(nc))
        const = ctx.enter_context(tc.tile_pool(name="const", bufs=1))
        wpool = ctx.enter_context(tc.tile_pool(name="wp", bufs=2))
        ohpool = ctx.enter_context(tc.tile_pool(name="ohp", bufs=3))
        epool = ctx.enter_context(tc.tile_pool(name="ep", bufs=1))
        psl = ctx.enter_context(tc.tile_pool(name="psl", bufs=3, space="PSUM"))
        psc = ctx.enter_context(tc.tile_pool(name="psc", bufs=2, space="PSUM"))
        pst = ctx.enter_context(tc.tile_pool(name="pst", bufs=1, space="PSUM"))
        pss = ctx.enter_context(tc.tile_pool(name="pss", bufs=1, space="PSUM"))
        stg = ctx.enter_context(tc.tile_pool(name="stg", bufs=3))
        stg2 = ctx.enter_context(tc.tile_pool(name="stg2", bufs=3))
        small = ctx.enter_context(tc.tile_pool(name="small", bufs=1))
        apool = ctx.enter_context(tc.tile_pool(name="ap", bufs=2))
        dram = ctx.enter_context(tc.tile_pool(name="dram", bufs=1, space="DRAM"))
        bvpool = (
            ctx.enter_context(tc.tile_pool(name="bvp", bufs=2)) if has_bias else None
        )

        xk = const.tile([P, KH, RT], F32R)
        nc.scalar.dma_start(
            xk[:], xT[:, :].rearrange("(k p) t -> p k t", p=P).bitcast(F32R)
        )
        wgt = const.tile([P, KH, 2], F32R)
        nc.sync.dma_start(
            wgt[:], wg[:, :].rearrange("(k p) o -> p k o", p=P).bitcast(F32R)
        )
        bgt = const.tile([P, 1], F32)
        nc.sync.dma_start(bgt[:], bg[:, :])
        ident = const.tile([P, P], F32R)
        nc.scalar.dma_start(ident[:], idn[:, :].bitcast(F32R))
        encsel_sb = const.tile([P, B], F32)
        nc.sync.dma_start(encsel_sb[:], encsel[:, :])
        selT_sb = const.tile([P, B * KL, P], F32R)
        nc.scalar.dma_start(
            selT_sb[:],
            selT[:, :, :].rearrange("b (c p) s -> p (b c) s", p=P).bitcast(F32R),
        )
        cdTsel = const.tile([P, B, T], F32R)
        iota_sb = const.tile([P, VS], F32)
        nc.gpsimd.iota(
            iota_sb[:],
            pattern=[[1, VS]],
            base=0,
            channel_multiplier=0,
            allow_small_or_imprecise_dtypes=True,
        )

        # ---- p_gen for all batches: p_all [P, B] ----
        p_all = small.tile([P, B], F32)
        for m in range(B):
            ps_p = pss.tile([P, 2], F32)
            for k in range(KH):
                nc.tensor.matmul(
                    ps_p[:T, :2],
                    lhsT=xk[:, k, bass.ts(m, T)],
                    rhs=wgt[:, k, :],
                    start=(k == 0),
                    stop=(k == KH - 1),
                )
            nc.scalar.activation(
                p_all[:T, m : m + 1], ps_p[:T, :1], AF.Sigmoid, bias=bgt[:T]
            )
        q_all = small.tile([P, B], F32)  # 1 - p
        nc.scalar.activation(q_all[:T], p_all[:T], AF.Identity, bias=1.0, scale=-1.0)

        # ---- copy distributions, transposed, per batch ----
        cdT = const.tile([P, B * KL, P], F32R)
        sa = small.tile([P, B], F32)
        for m in range(B):
            at = apool.tile([P, L], F32)
            nc.vector.memset(at[:], 0.0)
            nc.scalar.dma_start(at[:T, :], attn[bass.ts(m, T), :])
            ea = apool.tile([P, L], F32, tag="ea")
            nc.vector.memset(ea[:], 0.0)
            nc.scalar.activation(
                ea[:T], at[:T], AF.Exp, accum_out=sa[:T, m : m + 1]
            )
            ra = apool.tile([P, 1], F32, tag="ra")
            nc.vector.reciprocal(ra[:T], sa[:T, m : m + 1])
            qr = apool.tile([P, 1], F32, tag="qr")
            nc.vector.memset(qr[:], 0.0)
            nc.vector.tensor_tensor(qr[:T], q_all[:T, m : m + 1], ra[:T], op=ALU.mult)
            cd = apool.tile([P, L], F32R, tag="cd")
            nc.vector.tensor_scalar(cd[:], ea[:], qr[:, :1], None, op0=ALU.mult)
            for c in range(KL):
                ps_t = pst.tile([P, P], F32R)
                nc.tensor.transpose(ps_t[:], cd[:, bass.ts(c, P)], ident[:])
                nc.vector.tensor_copy(cdT[:, m * KL + c, :], ps_t[:])
            ps_s = pss.tile([P, T], F32, tag="sel")
            for c in range(KL):
                nc.tensor.matmul(
                    ps_s[:, :],
                    lhsT=selT_sb[:, m * KL + c, :],
                    rhs=cdT[:, m * KL + c, :T],
                    start=(c == 0),
                    stop=(c == KL - 1),
                )
            nc.vector.tensor_copy(cdTsel[:, m, :], ps_s[:, :])

        # ---- phase A+B in two pipelined groups of 4 batches: each group's
        # softmax-denominator AllReduce and phase B overlap the other group's
        # phase A (W chunk tiles are re-loaded per group; 2 x 16 MB).
        sparts = small.tile([P, B * NCJ], F32)
        E = epool.tile([P, B, NCJ, NC], BF16)
        GB = B // 2
        for g in range(2):
            ms = range(g * GB, (g + 1) * GB)
            for j in range(NCJ):
                wt = wpool.tile([P, KH, NC], F32R)
                nc.sync.dma_start(
                    wt[:],
                    w[:, bass.ts(j, NC)]
                    .rearrange("(k p) n -> p k n", p=P)
                    .bitcast(F32R),
                )
                bvt = None
                if has_bias:
                    bvt = bvpool.tile([P, NC], F32)
                    nc.sync.dma_start(bvt[:], bv[:, bass.ts(j, NC)])
                for m in ms:
                    ps_l = psl.tile([P, NC], F32)
                    for k in range(KH):
                        nc.tensor.matmul(
                            ps_l[:T, :],
                            lhsT=xk[:, k, bass.ts(m, T)],
                            rhs=wt[:, k, :],
                            start=(k == 0),
                            stop=(k == KH - 1),
                        )
                    if has_bias:
                        nc.vector.tensor_tensor(
                            ps_l[:T, :], ps_l[:T, :], bvt[:T, :], op=ALU.add
                        )
                    nc.scalar.activation(
                        E[:T, m, j, :],
                        ps_l[:T, :],
                        AF.Exp,
                        accum_out=sparts[:T, m * NCJ + j : m * NCJ + j + 1],
                    )

            # ---- group-local S, AllReduce across cores ----
            s_loc = small.tile([P, GB], F32, tag=f"sl{g}")
            nc.vector.memset(s_loc[:], 0.0)
            nc.vector.reduce_sum(
                s_loc[:T],
                sparts[:T, g * GB * NCJ : (g + 1) * GB * NCJ].rearrange(
                    "p (m j) -> p m j", m=GB
                ),
                axis=mybir.AxisListType.X,
            )
            cc_in = dram.tile([P, GB], F32, tag=f"ci{g}")
            cc_out = dram.tile([P, GB], F32, tag=f"co{g}")
            nc.gpsimd.dma_start(cc_in[:], s_loc[:])
            nc.gpsimd.collective_compute(
                "AllReduce",
                ALU.add,
                replica_groups=[list(range(NCORES))],
                ins=[cc_in[:].opt()],
                outs=[cc_out[:].opt()],
            )
            S_tot = small.tile([P, GB], F32, tag=f"st{g}")
            nc.gpsimd.dma_start(S_tot[:], cc_out[:])
            rS = small.tile([P, GB], F32, tag=f"rs{g}")
            nc.vector.reciprocal(rS[:T], S_tot[:T])
            pscale = small.tile([P, GB], F32, tag=f"pp{g}")
            nc.vector.tensor_tensor(
                pscale[:T], p_all[:T, g * GB : (g + 1) * GB], rS[:T], op=ALU.mult
            )

            # ---- phase B for this group ----
            for m in ms:
                for j in range(NCJ):
                    oht = ohpool.tile([P, NC], F32R)
                    nc.vector.tensor_scalar(
                        oht[:, :],
                        iota_sb[:, bass.ts(j, NC)],
                        encsel_sb[:, m : m + 1],
                        None,
                        op0=ALU.is_equal,
                    )
                    ps_c = psc.tile([P, NC], F32, tag="c")
                    nc.tensor.matmul(
                        ps_c[:T, :],
                        lhsT=cdTsel[:, m, :T],
                        rhs=oht[:, :],
                        start=True,
                        stop=True,
                    )
                    comb = stg.tile([P, NC], F32)
                    nc.vector.scalar_tensor_tensor(
                        comb[:T, :],
                        E[:T, m, j, :],
                        pscale[:T, m - g * GB : m - g * GB + 1],
                        ps_c[:T, :],
                        op0=ALU.mult,
                        op1=ALU.add,
                    )
                    res = stg2.tile([P, NC], F32)
                    nc.scalar.activation(res[:T, :], comb[:T, :], AF.Ln)
                    oeng = nc.scalar if j % 2 == 0 else nc.sync
                    oeng.dma_start(
                        out[bass.ts(m, T), bass.ts(j, NC)], res[:T, :]
                    )

    return _legalize_waits(nc)


SHARD_MODE = "vocab"  # "vocab" (tensor parallel) or "batch" (data parallel)


def prepare(x, attn_dist, enc_input, W_vocab, b_vocab, w_gen, b_gen, mode=None):
    """Build (nc, in_maps, assemble_fn) for the chosen sharding mode."""
    mode = mode or SHARD_MODE
    x = np.ascontiguousarray(x, dtype=np.float32)
    attn_dist = np.ascontiguousarray(attn_dist, dtype=np.float32)
    enc_input = np.asarray(enc_input)
    W_vocab = np.ascontiguousarray(W_vocab, dtype=np.float32)
    b_vocab = np.asarray(b_vocab, dtype=np.float32)
    w_gen = np.ascontiguousarray(w_gen, dtype=np.float32)
    b_gen = np.asarray(b_gen, dtype=np.float32)

    has_bias = bool(np.any(b_vocab))
    bg_b = np.broadcast_to(b_gen.reshape(1, 1), (P, 1)).copy()
    wg2 = np.concatenate([w_gen, np.zeros_like(w_gen)], axis=1)

    if mode == "batch":
        key = ("batch", has_bias)
        if key not in _cache:
            _cache[key] = _build_batch_sharded(has_bias)
        nc = _cache[key]
        in_maps = []
        for b in range(B):
            oh = np.zeros((L, V), dtype=ml_dtypes.bfloat16)
            oh[np.arange(L), enc_input[b]] = 1
            m = {
                "xT": np.ascontiguousarray(x[b].T),
                "w": W_vocab,
                "attn": attn_dist[b],
                "ohT": oh,
                "wg": wg2,
                "bg": bg_b,
            }
            if has_bias:
                m["bv"] = np.broadcast_to(b_vocab.reshape(1, V), (P, V)).copy()
            in_maps.append(m)

        def assemble(outs):
            return np.stack(outs, axis=0)

        return nc, in_maps, assemble

    key = ("vocab", has_bias)
    if key not in _cache:
        _cache[key] = _build_vocab_sharded(has_bias)
    nc = _cache[key]

    xTall = np.ascontiguousarray(x.reshape(RT, H).T)
    attn_flat = attn_dist.reshape(RT, L)
    in_maps = []
    for k in range(NCORES):
        lo = k * VS
        enc_loc = (enc_input - lo).astype(np.int64)  # [B, L]
        encsel_sb = np.full((P, B), -1.0, dtype=np.float32)
        selT_h = np.zeros((B, L, P), dtype=np.float32)
        for b in range(B):
            sel = np.nonzero((enc_loc[b] >= 0) & (enc_loc[b] < VS))[0]
            if len(sel) > P:
                raise OverflowError("more than 128 in-shard indices")
            encsel_sb[: len(sel), b] = enc_loc[b, sel]
            selT_h[b, sel, np.arange(len(sel))] = 1.0
        m = {
            "xT": xTall,
            "w": np.ascontiguousarray(W_vocab[:, lo : lo + VS]),
            "attn": attn_flat,
            "encsel": encsel_sb,
            "selT": selT_h,
            "wg": wg2,
            "bg": bg_b,
            "idn": np.eye(P, dtype=np.float32),
        }
        if has_bias:
            m["bv"] = np.broadcast_to(
                b_vocab[lo : lo + VS].reshape(1, VS), (P, VS)
            ).copy()
        in_maps.append(m)

    def assemble(outs):
        return np.concatenate([o.reshape(B, T, VS) for o in outs], axis=2)

    return nc, in_maps, assemble


def kernel(x, attn_dist, enc_input, W_vocab, b_vocab, w_gen, b_gen):
    nc, in_maps, assemble = prepare(
        x, attn_dist, enc_input, W_vocab, b_vocab, w_gen, b_gen
    )
    res = run_bass_kernel_spmd(nc, in_maps, core_ids=list(range(NCORES)))
    return assemble([res.results[c]["out"] for c in range(NCORES)])



# revision 12
# speedup vs baseline: 1.4279x; 1.4279x over previous
"""Pointer-generator copy layer on 8 Trainium2 NeuronCores (tensor-parallel
over the vocab dim, VS=4000 per core).

Per row r=(b,t) the reference computes
  p      = sigmoid(x_r @ w_gen + b_gen)
  E_v    = exp(x_r @ W_v + bv_v);  S = sum_v E_v
  corr   = scatter-add of (1-p)*softmax(attn_r) into vocab slots
  out_v  = log(E_v * p/S + corr_v)

This kernel exploits log-space sparsity: corr_v is nonzero at <=127 unique
in-shard vocab columns per batch, so
  out_v = logit_v + c_r                                  (non-hit columns)
  out_v = logit_v + c_r + delta_s                        (hit column u_s)
with c_r = log(p/S) and delta_s = log(1 + corr_s * S/(p*E_s)).  The dense
log/softmax pass disappears: the Act engine only runs exp (for S), and the
per-column correction is a tiny [<=127, T] fixup scattered back into the
output by one PE matmul per 1000-wide chunk (one-hot built on DVE from a
host iota, with partition 127 an all-ones row that broadcasts c_r).

Matmuls run as fp8(e4m3) DoubleRow (2 contraction rows/cycle; W and x are
host-quantized, W scaled by 32 to stay in the e4m3 normal range, undone by
exp's scale=1/32).  The scatter/selection/transpose matmuls are fp16.  The
softmax denominator is combined across the 8 vocab shards with a tiny
AllGather per 4-batch group (cheaper than AllReduce in latency and
overlapped with the other group's main matmul phase).

All inputs are host-packed to their exact SBUF layouts (fp8/fp16) so every
DMA is wide and flat; per-core HBM traffic is ~19 MB (vs ~50 MB for the
f32 two-pass baseline).
"""

import numpy as np
import ml_dtypes
from contextlib import ExitStack

import concourse.bass as bass
import concourse.mybir as mybir
import concourse.tile as tile
from concourse.bass_utils import run_bass_kernel_spmd

B, T, H, V, L = 8, 100, 1024, 32000, 512
NCORES = 8
VS = V // NCORES          # 4000 vocab columns per core
RT = B * T                # 800 rows
P = 128
KT = H // 256             # 4 double-row k-tiles (256 contraction each)
NG = 4                    # 1000-wide granules per batch (PSUM: 2 banks each)
GW = VS // NG             # 1000
NSLOT = 128               # slot rows; slot 127 carries c_r (all-ones one-hot row)

F32 = mybir.dt.float32
F16 = mybir.dt.float16
BF16 = mybir.dt.bfloat16
F8 = mybir.dt.float8e4
AF = mybir.ActivationFunctionType
ALU = mybir.AluOpType
DR = mybir.MatmulPerfMode.DoubleRow

E4 = ml_dtypes.float8_e4m3
WSCALE = 32.0

_cache = {}


def _legalize_waits(nc, limit=1):
    """This walrus build accepts at most one sync-wait per instruction; the
    TileContext exit drain can carry several. Split extras onto NoOps."""
    for bb in nc.main_func.blocks:
        new_insts = []
        for ins in bb.instructions:
            si = ins.sync_info
            if si is not None and si.on_wait and len(si.on_wait) > limit:
                waits = list(si.on_wait)
                extra, keep = waits[:-limit], waits[-limit:]
                for k, w in enumerate(extra):
                    new_insts.append(
                        mybir.InstNoOp(
                            name=f"{ins.name}-ws{k}",
                            sync_info=mybir.SyncInfo(on_wait=[w], on_update=[]),
                            bass_nofuse=True,
                            engine=ins.engine,
                        )
                    )
                ins.sync_info = mybir.SyncInfo(
                    on_wait=keep, on_update=list(si.on_update)
                )
            new_insts.append(ins)
        bb.instructions[:] = new_insts
    return nc


def _build(has_bias: bool):
    nc = bass.Bass()
    wq_d = nc.dram_tensor("wq_d", [P, KT, 2, VS], F8, kind="ExternalInput")
    xq_d = nc.dram_tensor("xq_d", [P, KT, 2, RT], F8, kind="ExternalInput")
    wg_d = nc.dram_tensor("wg_d", [P, KT, 2, 16], F8, kind="ExternalInput")
    ws_d = nc.dram_tensor("ws_d", [P, KT, 2, B, NSLOT], F8, kind="ExternalInput")
    at_d = nc.dram_tensor("at_d", [P, B, L], F16, kind="ExternalInput")
    st_d = nc.dram_tensor("st_d", [P, B, L // P, NSLOT], F16, kind="ExternalInput")
    io_d = nc.dram_tensor("io_d", [P, GW], F16, kind="ExternalInput")
    es_d = nc.dram_tensor("es_d", [P, B * NG], F32, kind="ExternalInput")
    i16_d = nc.dram_tensor("i16_d", [P, P], F16, kind="ExternalInput")
    bg_d = nc.dram_tensor("bg_d", [P, 1], F32, kind="ExternalInput")
    bv_d = (
        nc.dram_tensor("bv_d", [P, VS], F32, kind="ExternalInput")
        if has_bias
        else None
    )
    bs_d = (
        nc.dram_tensor("bs_d", [P, B, P], F32, kind="ExternalInput")
        if has_bias
        else None
    )
    out = nc.dram_tensor("out", [RT, VS], F32, kind="ExternalOutput")

    with ExitStack() as ctx:
        tc = ctx.enter_context(tile.TileContext(nc))
        const = ctx.enter_context(tc.tile_pool(name="const", bufs=1))
        psl = ctx.enter_context(tc.tile_pool(name="psl", bufs=3, space="PSUM"))
        pss = ctx.enter_context(tc.tile_pool(name="pss", bufs=2, space="PSUM"))
        apool = ctx.enter_context(tc.tile_pool(name="ap", bufs=2))
        epool = ctx.enter_context(tc.tile_pool(name="ep", bufs=2))
        ohpool = ctx.enter_context(tc.tile_pool(name="ohp", bufs=3))
        stg = ctx.enter_context(tc.tile_pool(name="stg", bufs=3))
        fix = ctx.enter_context(tc.tile_pool(name="fix", bufs=2))
        dram = ctx.enter_context(tc.tile_pool(name="dram", bufs=1, space="DRAM"))

        # ---- resident inputs (host-packed layouts; flat, wide DMAs) ----
        iota = const.tile([P, GW], F16)
        nc.sync.dma_start(iota[:], io_d[:, :])
        encsh = const.tile([P, B * NG], F32)
        nc.sync.dma_start(encsh[:], es_d[:, :])
        bg = const.tile([P, 1], F32)
        nc.sync.dma_start(bg[:], bg_d[:, :])
        wgen = const.tile([P, KT, 2, 16], F8)
        nc.sync.dma_start(wgen[:], wg_d[:, :, :, :])
        idn16 = const.tile([P, P], F16)
        nc.scalar.dma_start(idn16[:], i16_d[:, :])
        xq = const.tile([P, KT, 2, RT], F8)
        nc.scalar.dma_start(xq[:], xq_d[:, :, :, :])
        attn = const.tile([P, B, L], F16)
        nc.sync.dma_start(attn[:], at_d[:, :, :])
        selT = const.tile([P, B, L // P, NSLOT], F16)
        nc.scalar.dma_start(selT[:], st_d[:, :, :, :])
        wsel = const.tile([P, KT, 2, B, NSLOT], F8)
        nc.sync.dma_start(wsel[:], ws_d[:, :, :, :, :])
        wq = const.tile([P, KT, 2, VS], F8)
        for q in range(4):
            eng = nc.sync if q % 2 == 0 else nc.scalar
            eng.dma_start(
                wq[:, :, :, bass.ts(q, GW)], wq_d[:, :, :, bass.ts(q, GW)]
            )
        if has_bias:
            bvt = const.tile([P, VS], F32)
            nc.scalar.dma_start(bvt[:], bv_d[:, :])
            bvsel = const.tile([P, B, P], F32)
            nc.sync.dma_start(bvsel[:], bs_d[:, :, :])

        # ---- persistent working state ----
        logits = const.tile([P, B, NG, GW], F16)      # true-scale logits
        cdT = const.tile([P, B, L // P, P], F16)      # copy-dist transposed
        corr = const.tile([P, B, P], F16)             # [row, slot] copy mass
        lsel = const.tile([P, B, P], F16)             # [row, slot] logits
        deltaT = const.tile([P, B, T], F16)           # [slot, row] deltas + c_r
        p_all = const.tile([P, B], F32)
        q_all = const.tile([P, B], F32)
        sa = const.tile([P, B], F32)
        sparts = const.tile([P, B * NG], F32)

        # ---- p_gen = sigmoid(x @ w_gen / 32 + b_gen) ----
        for m in range(B):
            ps_p = pss.tile([P, P], F32, tag="sp")
            for kt in range(KT):
                nc.tensor.matmul(
                    ps_p[:T, :16],
                    lhsT=xq[:, kt, :, bass.ts(m, T)],
                    rhs=wgen[:, kt, :, :],
                    start=(kt == 0),
                    stop=(kt == KT - 1),
                    perf_mode=DR,
                )
            nc.scalar.activation(
                p_all[:T, m : m + 1],
                ps_p[:T, :1],
                AF.Sigmoid,
                scale=1.0 / WSCALE,
                bias=bg[:T],
            )
        nc.scalar.activation(q_all[:T], p_all[:T], AF.Identity, bias=1.0, scale=-1.0)

        # ---- copy distributions: cd = (1-p)*softmax(attn); transpose; select ----
        for m in range(B):
            ea = apool.tile([P, L], F32, tag="ea")
            nc.vector.memset(ea[:], 0.0)
            nc.scalar.activation(
                ea[:T], attn[:T, m, :], AF.Exp, accum_out=sa[:T, m : m + 1]
            )
            ra = apool.tile([P, 1], F32, tag="ra")
            nc.vector.reciprocal(ra[:T], sa[:T, m : m + 1])
            qr = apool.tile([P, 1], F32, tag="qr")
            nc.vector.memset(qr[:], 0.0)
            nc.vector.tensor_tensor(qr[:T], q_all[:T, m : m + 1], ra[:T], op=ALU.mult)
            cd16 = apool.tile([P, L], F16, tag="cd")
            nc.vector.tensor_scalar(cd16[:], ea[:], qr[:, :1], None, op0=ALU.mult)
            for c in range(L // P):
                ps_t = pss.tile([P, P], F16, tag="sp")
                nc.tensor.transpose(ps_t[:], cd16[:, bass.ts(c, P)], idn16[:])
                nc.gpsimd.tensor_copy(cdT[:, m, c, :], ps_t[:])
            ps_s = pss.tile([P, P], F32, tag="sp")
            for c in range(L // P):
                nc.tensor.matmul(
                    ps_s[:T, :],
                    lhsT=cdT[:, m, c, :T],
                    rhs=selT[:, m, c, :],
                    start=(c == 0),
                    stop=(c == L // P - 1),
                )
            nc.vector.tensor_copy(corr[:T, m, :], ps_s[:T, :])

        # ---- per-slot logits, row-major: lsel[r, s] = (x_r . W_sel_s) / 32 ----
        for m in range(B):
            ps_ls = pss.tile([P, P], F32, tag="sp")
            for kt in range(KT):
                nc.tensor.matmul(
                    ps_ls[:T, :],
                    lhsT=xq[:, kt, :, bass.ts(m, T)],
                    rhs=wsel[:, kt, :, m, :],
                    start=(kt == 0),
                    stop=(kt == KT - 1),
                    perf_mode=DR,
                )
            if has_bias:
                nc.vector.tensor_tensor(
                    ps_ls[:T, :], ps_ls[:T, :], bvsel[:T, m, :], op=ALU.add
                )
            nc.vector.tensor_scalar(
                lsel[:T, m, :], ps_ls[:T, :], 1.0 / WSCALE, None, op0=ALU.mult
            )

        # ---- two groups of 4 batches: matmul+exp+drain, AllGather, fixup, out ----
        GB = B // 2
        for g in range(2):
            ms = range(g * GB, (g + 1) * GB)

            # phase A: logits granules, exp for the softmax denominator.
            # PSUM granule is [P, 2, 512] (two banks) with 500 used columns
            # per bank so every matmul output stays inside one bank.
            for m in ms:
                for gr in range(NG):
                    ps = psl.tile([P, 2, 512], F32, tag="g")
                    for half in range(2):
                        for kt in range(KT):
                            nc.tensor.matmul(
                                ps[:T, half, :500],
                                lhsT=xq[:, kt, :, bass.ts(m, T)],
                                rhs=wq[
                                    :, kt, :, gr * GW + half * 500 : gr * GW + (half + 1) * 500
                                ],
                                start=(kt == 0),
                                stop=(kt == KT - 1),
                                perf_mode=DR,
                            )
                    lg_v = logits[:T, m, gr, :].rearrange("p (h w) -> p h w", h=2)
                    if has_bias:
                        nc.vector.tensor_tensor(
                            ps[:T, :, :500],
                            ps[:T, :, :500],
                            bvt[:T, bass.ts(gr, GW)].rearrange(
                                "p (h w) -> p h w", h=2
                            ),
                            op=ALU.add,
                        )
                    esc = epool.tile([P, 2, 500], BF16, tag="esc")
                    nc.scalar.activation(
                        esc[:T],
                        ps[:T, :, :500],
                        AF.Exp,
                        scale=1.0 / WSCALE,
                        accum_out=sparts[:T, m * NG + gr : m * NG + gr + 1],
                    )
                    if gr % 2 == 0:
                        nc.vector.tensor_scalar(
                            lg_v, ps[:T, :, :500], 1.0 / WSCALE, None, op0=ALU.mult
                        )
                    else:
                        nc.gpsimd.tensor_scalar(
                            lg_v, ps[:T, :, :500], 1.0 / WSCALE, None, op0=ALU.mult
                        )

            # group-local denominator partials -> AllGather -> total S
            s_loc = const.tile([P, GB], F32, tag=f"sl{g}", name=f"s_loc{g}")
            nc.vector.memset(s_loc[:], 0.0)
            nc.vector.reduce_sum(
                s_loc[:T],
                sparts[:T, g * GB * NG : (g + 1) * GB * NG].rearrange(
                    "p (m r) -> p m r", m=GB
                ),
                axis=mybir.AxisListType.X,
            )
            cc_in = dram.tile([P, GB], F32, tag=f"ci{g}", name=f"cc_in{g}")
            cc_out = dram.tile([NCORES * P, GB], F32, tag=f"co{g}", name=f"cc_out{g}")
            nc.gpsimd.dma_start(cc_in[:], s_loc[:])
            nc.gpsimd.collective_compute(
                "AllGather",
                ALU.bypass,
                replica_groups=[list(range(NCORES))],
                ins=[cc_in[:].opt()],
                outs=[cc_out[:].opt()],
            )
            sg = const.tile([P, GB, NCORES], F32, tag=f"sg{g}", name=f"sg{g}")
            nc.gpsimd.dma_start(
                sg[:], cc_out[:, :].rearrange("(c p) m -> p m c", p=P)
            )
            s_tot = const.tile([P, GB], F32, tag=f"st{g}", name=f"s_tot{g}")
            nc.vector.reduce_sum(s_tot[:T], sg[:T, :, :], axis=mybir.AxisListType.X)
            r_tot = const.tile([P, GB], F32, tag=f"rt{g}", name=f"r_tot{g}")
            nc.vector.reciprocal(r_tot[:T], s_tot[:T])
            psc = const.tile([P, GB], F32, tag=f"pg{g}", name=f"psc{g}")  # p/S
            nc.vector.tensor_tensor(
                psc[:T], p_all[:T, g * GB : (g + 1) * GB], r_tot[:T], op=ALU.mult
            )
            crg = const.tile([P, GB], F32, tag=f"cg{g}", name=f"crg{g}")  # log(p/S)
            nc.scalar.activation(crg[:T], psc[:T], AF.Ln)

            # per-batch fixup, row-major [T, 128]:
            #   delta = log(1 + corr / (p/S * exp(lsel)));  col 127 <- c_r
            for mloc, m in enumerate(ms):
                esel = fix.tile([P, P], F32, tag="es")
                nc.scalar.activation(esel[:T], lsel[:T, m, :], AF.Exp)
                pe = fix.tile([P, P], F32, tag="pe")
                nc.vector.tensor_scalar(
                    pe[:T], esel[:T], psc[:T, mloc : mloc + 1], None, op0=ALU.mult
                )
                r1 = fix.tile([P, P], F32, tag="r1")
                nc.vector.reciprocal(r1[:T], pe[:T])
                rat = fix.tile([P, P], F32, tag="ra")
                nc.vector.tensor_tensor(rat[:T], corr[:T, m, :], r1[:T], op=ALU.mult)
                nc.vector.tensor_scalar(rat[:T], rat[:T], 1.0, None, op0=ALU.add)
                drm = fix.tile([P, P], F16, tag="dF")
                nc.vector.memset(drm[:], 0.0)
                nc.scalar.activation(drm[:T, :127], rat[:T, :127], AF.Ln)
                nc.vector.tensor_copy(drm[:T, 127:128], crg[:T, mloc : mloc + 1])
                ps_dt = pss.tile([P, P], F16, tag="sp")
                nc.tensor.transpose(ps_dt[:], drm[:], idn16[:])
                nc.vector.tensor_copy(deltaT[:, m, :], ps_dt[:, :T])

            # phase B: one-hot scatter of (delta, c_r) + final add + store
            for m in ms:
                for gr in range(NG):
                    oht = ohpool.tile([P, GW], F16, tag="oh")
                    nc.vector.tensor_scalar(
                        oht[:],
                        iota[:],
                        encsh[:, m * NG + gr : m * NG + gr + 1],
                        None,
                        op0=ALU.is_equal,
                    )
                    ps_c = psl.tile([P, 2, 512], F32, tag="g")
                    for half in range(2):
                        nc.tensor.matmul(
                            ps_c[:T, half, :500],
                            lhsT=deltaT[:, m, :T],
                            rhs=oht[:, half * 500 : (half + 1) * 500],
                            start=True,
                            stop=True,
                        )
                    lg_v = logits[:T, m, gr, :].rearrange("p (h w) -> p h w", h=2)
                    res = stg.tile([P, 2, 500], F32, tag="res")
                    if gr % 2 == 0:
                        nc.vector.tensor_tensor(
                            res[:T], lg_v, ps_c[:T, :, :500], op=ALU.add
                        )
                    else:
                        nc.gpsimd.scalar_tensor_tensor(
                            res[:T],
                            lg_v,
                            1.0,
                            ps_c[:T, :, :500],
                            op0=ALU.mult,
                            op1=ALU.add,
                        )
                    oeng = nc.sync if gr % 2 == 0 else nc.scalar
                    oeng.dma_start(
                        out[bass.ts(m, T), bass.ts(gr, GW)],
                        res[:T].rearrange("p h w -> p (h w)"),
                    )

    return _legalize_waits(nc)


def prepare(x, attn_dist, enc_input, W_vocab, b_vocab, w_gen, b_gen, mode=None):
    """Build (nc, in_maps, assemble_fn)."""
    x = np.ascontiguousarray(x, dtype=np.float32)
    attn_dist = np.ascontiguousarray(attn_dist, dtype=np.float32)
    enc_input = np.asarray(enc_input)
    W_vocab = np.ascontiguousarray(W_vocab, dtype=np.float32)
    b_vocab = np.asarray(b_vocab, dtype=np.float32)
    w_gen = np.ascontiguousarray(w_gen, dtype=np.float32)
    b_gen = np.asarray(b_gen, dtype=np.float32)

    has_bias = bool(np.any(b_vocab))
    key = ("v3", has_bias)
    if key not in _cache:
        _cache[key] = _build(has_bias)
    nc = _cache[key]

    def pack_k(a):
        # [H, N] -> [P, KT, 2, N] with h = kt*256 + i*128 + p
        n = a.shape[1]
        return np.ascontiguousarray(
            a.reshape(KT, 2, P, n).transpose(2, 0, 1, 3)
        )

    xT = x.reshape(RT, H).T                      # [H, RT]
    xq = pack_k(xT.astype(E4).astype(np.float32)).astype(E4)
    wg = np.zeros((H, 16), dtype=np.float32)
    wg[:, 0] = w_gen[:, 0] * WSCALE
    wgq = pack_k(wg).astype(E4)

    at16 = np.zeros((P, B, L), dtype=np.float16)
    at16[:T] = attn_dist.transpose(1, 0, 2)

    iota = np.broadcast_to(
        np.arange(GW, dtype=np.float32), (P, GW)
    ).copy()
    iota[127, :] = -2.0
    io16 = iota.astype(np.float16)

    i16 = np.eye(P, dtype=np.float16)
    bg = np.broadcast_to(b_gen.reshape(1, 1), (P, 1)).astype(np.float32).copy()

    in_maps = []
    for k in range(NCORES):
        lo = k * VS
        wsh = W_vocab[:, lo : lo + VS] * WSCALE
        wq8 = pack_k(wsh).astype(E4)            # [P, KT, 2, VS]

        es = np.full((P, B * NG), -1.0, dtype=np.float32)
        es[127, :] = -2.0
        st16 = np.zeros((P, B, L // P, NSLOT), dtype=np.float16)
        ws8 = np.zeros((P, KT, 2, B, NSLOT), dtype=E4)
        bs = np.zeros((P, B, P), dtype=np.float32)
        for m in range(B):
            eloc = enc_input[m].astype(np.int64) - lo
            sel = np.nonzero((eloc >= 0) & (eloc < VS))[0]
            u = np.unique(eloc[sel])
            if len(u) > NSLOT - 1:
                raise OverflowError("more than 127 unique in-shard indices")
            slot_of = {v: s for s, v in enumerate(u)}
            for lpos in sel:
                s = slot_of[eloc[lpos]]
                st16[lpos % P, m, lpos // P, s] = 1.0
            for gr in range(NG):
                shifted = u - gr * GW
                inr = (shifted >= 0) & (shifted < GW)
                es[: len(u), m * NG + gr] = np.where(inr, shifted, -1.0)
            ws8[:, :, :, m, : len(u)] = wq8[:, :, :, u]
            if has_bias:
                bs[:, m, : len(u)] = (b_vocab[lo + u] * WSCALE)[None, :]

        mmap = {
            "wq_d": wq8,
            "xq_d": xq,
            "wg_d": wgq,
            "ws_d": ws8,
            "at_d": at16,
            "st_d": st16,
            "io_d": io16,
            "es_d": es,
            "i16_d": i16,
            "bg_d": bg,
        }
        if has_bias:
            mmap["bv_d"] = np.broadcast_to(
                b_vocab[lo : lo + VS].reshape(1, VS), (P, VS)
            ).astype(np.float32).copy()
            mmap["bs_d"] = bs
        in_maps.append(mmap)

    def assemble(outs):
        return np.concatenate([o.reshape(B, T, VS) for o in outs], axis=2)

    return nc, in_maps, assemble


def kernel(x, attn_dist, enc_input, W_vocab, b_vocab, w_gen, b_gen):
    nc, in_maps, assemble = prepare(
        x, attn_dist, enc_input, W_vocab, b_vocab, w_gen, b_gen
    )
    res = run_bass_kernel_spmd(nc, in_maps, core_ids=list(range(NCORES)))
    return assemble([res.results[c]["out"] for c in range(NCORES)])
